# revision 2
# baseline (speedup 1.0000x reference)
"""GPTQ 4-bit linear kernel for Trainium2, 8-core token-parallel SPMD.

Math:  out[m,n] = sum_k x[m,k] * W[k,n],  W = scale[g,n] * (q[k,n] - z[g,n] - 1),
       g = k // 128 (group size 128 == SBUF partition count).

Decomposition: W = scale*q - scale*(z+1), so
    out = x @ (scale*q)  +  S @ zc
with S[m,g] = sum_{k in g} x[m,k] (computed on-device via one-hot matmuls) and
zc[g,n] = -(scale[g,n]*(z[g,n]+1)) (host-prepared quantization constants).

Per core (token shard m of 1024):
 - x shipped transposed+row-permuted as fp16 xtp[k', m]; within every
   256-row block, rows are reordered evens-then-odds so that one 128-row
   byte-tile of packed qweight aligns with one 128-partition weight tile.
 - qweight shipped as a byte-transposed uint8 array qb[k//2, n]; on-chip
   nibble extract (DVE bitvec) + scale multiply (DVE arith) produce fp16
   weight tiles; TensorE accumulates 32 k-tiles + 1 rank-32 zero-point
   correction matmul (fp32r) per PSUM tile.
 - output written as out.T [n, m] tiles; host reassembles/transposes.
"""

import numpy as np

import concourse.bass as bass
import concourse.tile as tile
import concourse.mybir as mybir
from concourse import bacc
from concourse.bass_utils import run_bass_kernel_spmd

NCORES = 8
B, SEQ, IN_F, OUT_F = 4, 2048, 4096, 4096
GS = 128
NG = IN_F // GS          # 32 groups
NT_K = IN_F // 128       # 32 k' tiles
M_TOT = B * SEQ          # 8192 tokens
M = M_TOT // NCORES      # 1024 tokens per core
NCH = 1024               # n columns per chunk (4 chunks)
F16 = mybir.dt.float16
F32 = mybir.dt.float32
F32R = mybir.dt.float32r
U8 = mybir.dt.uint8

_cache = {}


def _build(m=M):
    nc = bacc.Bacc("TRN2", target_bir_lowering=False, debug=False,
                   num_devices=NCORES)
    xtp = nc.dram_tensor("xtp", [IN_F, m], F16, kind="ExternalInput").ap()
    qb = nc.dram_tensor("qb", [IN_F // 2, OUT_F], U8, kind="ExternalInput").ap()
    scp = nc.dram_tensor("scp", [IN_F // 2, OUT_F], F32, kind="ExternalInput").ap()
    zc = nc.dram_tensor("zc", [NG, OUT_F], F32R, kind="ExternalInput").ap()
    gm = nc.dram_tensor("gm", [IN_F, NG], F16, kind="ExternalInput").ap()
    outT = nc.dram_tensor("outT", [OUT_F, m], F32, kind="ExternalOutput").ap()

    n_mh = m // 512       # m half-chunks of 512

    with tile.TileContext(nc) as tc:
        with tc.tile_pool(name="resident", bufs=1) as res:
            # resident activations: [128, t*m + m_local]
            xtp_sb = res.tile([128, NT_K * m], F16)
            for t in range(NT_K):
                nc.sync.dma_start(xtp_sb[:, bass.ts(t, m)],
                                  xtp[t * 128:(t + 1) * 128, :])
            gm_sb = res.tile([128, NT_K * NG], F16)
            for t in range(NT_K):
                nc.sync.dma_start(gm_sb[:, bass.ts(t, NG)],
                                  gm[t * 128:(t + 1) * 128, :])
            zc_sb = res.tile([NG, OUT_F], F32R)
            nc.sync.dma_start(zc_sb[:], zc)
            st_sb = res.tile([NG, m], F32R)

            # --- S phase: S.T[g, m] = sum_{k in g} xtp[k, m] via one-hot matmuls
            with tc.tile_pool(name="psS", bufs=1, space="PSUM") as psS_pool:
                psS = [psS_pool.tile([NG, 512], F32, tag=f"psS{i}", name=f"psS{i}")
                       for i in range(n_mh)]
                for t in range(NT_K):
                    for mc in range(n_mh):
                        nc.tensor.matmul(
                            psS[mc][:], gm_sb[:, bass.ts(t, NG)],
                            xtp_sb[:, bass.ds(t * m + mc * 512, 512)],
                            start=(t == 0), stop=(t == NT_K - 1))
                for mc in range(n_mh):
                    nc.scalar.copy(st_sb[:, bass.ts(mc, 512)], psS[mc][:])

            # --- main: per n-chunk of NCH columns
            with tc.tile_pool(name="wf", bufs=1) as wfp, \
                 tc.tile_pool(name="work", bufs=3) as work, \
                 tc.tile_pool(name="stage", bufs=4) as stage, \
                 tc.tile_pool(name="ps", bufs=1, space="PSUM") as psp:
                for nci in range(OUT_F // NCH):
                    n0 = nci * NCH
                    wfs = []
                    for p in range(16):
                        qb_t = work.tile([128, NCH], U8, tag="qb", name=f"qb_{nci}_{p}")
                        nc.sync.dma_start(
                            qb_t[:], qb[p * 128:(p + 1) * 128, n0:n0 + NCH])
                        sc_t = work.tile([128, NCH], F32, tag="sc", name=f"sc_{nci}_{p}")
                        nc.sync.dma_start(
                            sc_t[:], scp[p * 128:(p + 1) * 128, n0:n0 + NCH])
                        l8 = work.tile([128, NCH], U8, tag="l8", name=f"l8_{nci}_{p}")
                        nc.vector.tensor_scalar(l8[:], qb_t[:], 0xF, None,
                                                op0=mybir.AluOpType.bitwise_and)
                        h8 = work.tile([128, NCH], U8, tag="h8", name=f"h8_{nci}_{p}")
                        nc.vector.tensor_scalar(h8[:], qb_t[:], 4, None,
                                                op0=mybir.AluOpType.logical_shift_right)
                        wfe = wfp.tile([128, NCH], F16, tag=f"wf{2*p}",
                                       name=f"wf_{nci}_{2*p}")
                        nc.vector.tensor_tensor(wfe[:], l8[:], sc_t[:],
                                                op=mybir.AluOpType.mult)
                        wfo = wfp.tile([128, NCH], F16, tag=f"wf{2*p+1}",
                                       name=f"wf_{nci}_{2*p+1}")
                        nc.vector.tensor_tensor(wfo[:], h8[:], sc_t[:],
                                                op=mybir.AluOpType.mult)
                        wfs += [wfe, wfo]

                    for mh in range(n_mh):
                        m0 = mh * 512
                        for nt in range(NCH // 128):
                            ps = psp.tile([128, 512], F32, tag=f"ps{nt}",
                                          name=f"ps_{nci}_{mh}_{nt}")
                            for t in range(NT_K):
                                nc.tensor.matmul(
                                    ps[:], wfs[t][:, bass.ts(nt, 128)],
                                    xtp_sb[:, bass.ds(t * m + m0, 512)],
                                    start=(t == 0), stop=False)
                            nc.tensor.matmul(
                                ps[:], zc_sb[:, bass.ds(n0 + nt * 128, 128)],
                                st_sb[:, bass.ds(m0, 512)],
                                start=False, stop=True)
                            stg = stage.tile([128, 512], F32, tag="stg",
                                             name=f"stg_{nci}_{mh}_{nt}")
                            nc.scalar.copy(stg[:], ps[:])
                            nc.sync.dma_start(
                                outT[n0 + nt * 128:n0 + (nt + 1) * 128,
                                     m0:m0 + 512],
                                stg[:])
    nc.compile()
    return nc


def _build_null(m=M):
    """Same I/O surface as _build, near-zero device work (for differential timing)."""
    nc = bacc.Bacc("TRN2", target_bir_lowering=False, debug=False,
                   num_devices=NCORES)
    xtp = nc.dram_tensor("xtp", [IN_F, m], F16, kind="ExternalInput").ap()
    nc.dram_tensor("qb", [IN_F // 2, OUT_F], U8, kind="ExternalInput")
    nc.dram_tensor("scp", [IN_F // 2, OUT_F], F32, kind="ExternalInput")
    nc.dram_tensor("zc", [NG, OUT_F], F32R, kind="ExternalInput")
    nc.dram_tensor("gm", [IN_F, NG], F16, kind="ExternalInput")
    outT = nc.dram_tensor("outT", [OUT_F, m], F32, kind="ExternalOutput").ap()
    with tile.TileContext(nc) as tc:
        with tc.tile_pool(name="p", bufs=1) as pool:
            t = pool.tile([128, 128], F16)
            nc.sync.dma_start(t[:], xtp[0:128, 0:128])
            o = pool.tile([128, 128], F32)
            nc.vector.tensor_copy(o[:], t[:])
            nc.sync.dma_start(outT[0:128, 0:128], o[:])
    nc.compile()
    return nc


def _prep(x, qweight, qzeros, scales, m=M, ncores=NCORES):
    """Host-side layout marshaling -> per-core input maps."""
    # activations: transpose + evens-then-odds permutation within 256-blocks
    x2 = np.ascontiguousarray(x.reshape(M_TOT, IN_F))
    perm = np.empty(IN_F, dtype=np.int64)
    for t in range(NT_K):
        P, par = divmod(t, 2)
        perm[t * 128:(t + 1) * 128] = 256 * P + 2 * np.arange(128) + par
    xtp = np.ascontiguousarray(x2.T[perm]).astype(np.float16)  # [IN_F, M_TOT]

    # packed weights as byte rows: qb[k//2, n] = byte holding nibbles (2bk, 2bk+1)
    qb = np.ascontiguousarray(
        qweight.view(np.uint8).reshape(IN_F // 8, OUT_F, 4)
        .transpose(0, 2, 1).reshape(IN_F // 2, OUT_F))

    # scales broadcast to byte-row layout: row r -> group r//64
    scp = np.ascontiguousarray(np.repeat(scales, 64, axis=0).astype(np.float32))

    # zero-point correction constants zc[g,n] = -(scale*(z+1))
    u = qzeros.view(np.uint32)
    shifts = (4 * np.arange(8, dtype=np.uint32))[None, None, :]
    z = ((u[:, :, None] >> shifts) & np.uint32(0xF)).reshape(NG, OUT_F)
    zcv = np.ascontiguousarray((-(scales.astype(np.float64)
                                  * (z.astype(np.float64) + 1.0))).astype(np.float32))

    # one-hot group map in permuted k' order: gm[k', g] = 1 if group(k') == g
    gmv = np.zeros((IN_F, NG), dtype=np.float16)
    rows = np.arange(IN_F)
    t_idx = rows // 128
    p_idx = rows % 128
    g_idx = 2 * (t_idx // 2) + p_idx // 64
    gmv[rows, g_idx] = 1.0

    in_maps = []
    for c in range(ncores):
        in_maps.append({
            "xtp": np.ascontiguousarray(xtp[:, c * m:(c + 1) * m]),
            "qb": qb, "scp": scp, "zc": zcv, "gm": gmv,
        })
    return in_maps


def kernel(x, qweight, qzeros, scales):
    if "nc" not in _cache:
        _cache["nc"] = _build()
    nc = _cache["nc"]
    in_maps = _prep(x, qweight, qzeros, scales)
    res = run_bass_kernel_spmd(nc, in_maps, core_ids=list(range(NCORES)))
    outs = [r["outT"] for r in res.results]          # each [OUT_F, M]
    full = np.concatenate(outs, axis=1)              # [OUT_F, M_TOT]
    return np.ascontiguousarray(full.T).reshape(B, SEQ, OUT_F).astype(np.float32)


# revision 13
# speedup vs baseline: 1004.0121x; 1004.0121x over previous
"""GPTQ 4-bit linear kernel for Trainium2, 8-core token-parallel SPMD.

Math:  out[m,n] = sum_k x[m,k] * W[k,n],  W = scale[g,n] * (q[k,n] - z[g,n] - 1),
       g = k // 128 (group size 128 == SBUF partition count).

Decomposition: W = scale*q - scale*(z+1), so
    out = x @ (scale*q)  +  S @ zc
with S[m,g] = sum_{k in g} x[m,k] (computed on-device via one-hot matmuls) and
zc[g,n] = -(scale[g,n]*(z[g,n]+1)) (host-prepared quantization constants).

Per core (token shard m of 1024):
 - x shipped transposed+row-permuted as fp16 xtp[k', m]; within every
   256-row block, rows are reordered evens-then-odds so that one 128-row
   byte-tile of packed qweight aligns with one 128-partition weight tile.
 - qweight shipped as a byte-transposed uint8 array qb[k//2, n]; on-chip
   nibble extract (DVE bitvec) + scale multiply (DVE arith) produce fp16
   weight tiles; TensorE accumulates 32 k-tiles + 1 rank-32 zero-point
   correction matmul (fp32r) per PSUM tile.
 - output written as out.T [n, m] tiles; host reassembles/transposes.
"""

import numpy as np

import concourse.bass as bass
import concourse.tile as tile
import concourse.mybir as mybir
from concourse import bacc
from concourse.bass_utils import run_bass_kernel_spmd

NCORES = 8
B, SEQ, IN_F, OUT_F = 4, 2048, 4096, 4096
GS = 128
NG = IN_F // GS          # 32 groups
NT_K = IN_F // 128       # 32 k' tiles
M_TOT = B * SEQ          # 8192 tokens
M = M_TOT // NCORES      # 1024 tokens per core
NCH = 1024               # n columns per chunk (4 chunks)
F16 = mybir.dt.float16
F32 = mybir.dt.float32
F32R = mybir.dt.float32r
U8 = mybir.dt.uint8

_cache = {}


def _build(m=M, iters=1):
    nc = bacc.Bacc("TRN2", target_bir_lowering=False, debug=False,
                   num_devices=NCORES)
    xtp = nc.dram_tensor("xtp", [IN_F, m], F16, kind="ExternalInput").ap()
    qb = nc.dram_tensor("qb", [IN_F // 2, OUT_F], U8, kind="ExternalInput").ap()
    scl = nc.dram_tensor("scl", [NG, OUT_F], F32, kind="ExternalInput").ap()
    zc = nc.dram_tensor("zc", [NG, OUT_F], F32R, kind="ExternalInput").ap()
    gm = nc.dram_tensor("gm", [IN_F, NG], F16, kind="ExternalInput").ap()
    outT = nc.dram_tensor("outT", [OUT_F, m], F32, kind="ExternalOutput").ap()

    n_mh = m // 512       # m half-chunks of 512

    with tile.TileContext(nc) as tc:
        with tc.tile_pool(name="resident", bufs=1) as res:
            # resident activations: [128, t*m + m_local]
            xtp_sb = res.tile([128, NT_K * m], F16)
            for t in range(NT_K):
                nc.sync.dma_start(xtp_sb[:, bass.ts(t, m)],
                                  xtp[t * 128:(t + 1) * 128, :])
            gm_sb = res.tile([128, NT_K * NG], F16)
            for t in range(NT_K):
                nc.sync.dma_start(gm_sb[:, bass.ts(t, NG)],
                                  gm[t * 128:(t + 1) * 128, :])
            zc_sb = res.tile([NG, OUT_F], F32R)
            nc.sync.dma_start(zc_sb[:], zc)
            st_sb = res.tile([NG, m], F32R)

            # --- S phase: S.T[g, m] = sum_{k in g} xtp[k, m] via one-hot matmuls
            with tc.tile_pool(name="psS", bufs=1, space="PSUM") as psS_pool:
                psS = [psS_pool.tile([NG, 512], F32, tag=f"psS{i}", name=f"psS{i}")
                       for i in range(n_mh)]
                for t in range(NT_K):
                    for mc in range(n_mh):
                        nc.tensor.matmul(
                            psS[mc][:], gm_sb[:, bass.ts(t, NG)],
                            xtp_sb[:, bass.ds(t * m + mc * 512, 512)],
                            start=(t == 0), stop=(t == NT_K - 1))
                for mc in range(n_mh):
                    nc.scalar.copy(st_sb[:, bass.ts(mc, 512)], psS[mc][:])

            # --- main: per n-chunk of NCH columns
            from contextlib import ExitStack
            _loop = ExitStack()
            if iters > 1:
                _loop.enter_context(tc.For_i(0, iters, 1))
            with tc.tile_pool(name="wf", bufs=1) as wfp, \
                 tc.tile_pool(name="work", bufs=3) as work, \
                 tc.tile_pool(name="stage", bufs=4) as stage, \
                 tc.tile_pool(name="ps", bufs=1, space="PSUM") as psp:
                for nci in range(OUT_F // NCH):
                    n0 = nci * NCH
                    wfs = []
                    for p in range(16):
                        qb_t = work.tile([128, NCH], U8, tag="qb", name=f"qb_{nci}_{p}")
                        nc.sync.dma_start(
                            qb_t[:], qb[p * 128:(p + 1) * 128, n0:n0 + NCH])
                        sc_t = work.tile([128, NCH], F32, tag="sc", name=f"sc_{nci}_{p}")
                        for j in range(2):
                            nc.sync.dma_start(
                                sc_t[64 * j:64 * (j + 1), :],
                                scl[2 * p + j, n0:n0 + NCH].partition_broadcast(64))
                        l8 = work.tile([128, NCH], U8, tag="l8", name=f"l8_{nci}_{p}")
                        nc.vector.tensor_scalar(l8[:], qb_t[:], 0xF, None,
                                                op0=mybir.AluOpType.bitwise_and)
                        h8 = work.tile([128, NCH], U8, tag="h8", name=f"h8_{nci}_{p}")
                        nc.vector.tensor_scalar(h8[:], qb_t[:], 4, None,
                                                op0=mybir.AluOpType.logical_shift_right)
                        wfe = wfp.tile([128, NCH], F16, tag=f"wf{2*p}",
                                       name=f"wf_{nci}_{2*p}")
                        nc.vector.tensor_tensor(wfe[:], l8[:], sc_t[:],
                                                op=mybir.AluOpType.mult)
                        wfo = wfp.tile([128, NCH], F16, tag=f"wf{2*p+1}",
                                       name=f"wf_{nci}_{2*p+1}")
                        nc.vector.tensor_tensor(wfo[:], h8[:], sc_t[:],
                                                op=mybir.AluOpType.mult)
                        wfs += [wfe, wfo]

                    for mh in range(n_mh):
                        m0 = mh * 512
                        for nt in range(NCH // 128):
                            ps = psp.tile([128, 512], F32, tag=f"ps{nt}",
                                          name=f"ps_{nci}_{mh}_{nt}")
                            for t in range(NT_K):
                                nc.tensor.matmul(
                                    ps[:], wfs[t][:, bass.ts(nt, 128)],
                                    xtp_sb[:, bass.ds(t * m + m0, 512)],
                                    start=(t == 0), stop=False)
                            nc.tensor.matmul(
                                ps[:], zc_sb[:, bass.ds(n0 + nt * 128, 128)],
                                st_sb[:, bass.ds(m0, 512)],
                                start=False, stop=True)
                            stg = stage.tile([128, 512], F32, tag="stg",
                                             name=f"stg_{nci}_{mh}_{nt}")
                            nc.scalar.copy(stg[:], ps[:])
                            nc.sync.dma_start(
                                outT[n0 + nt * 128:n0 + (nt + 1) * 128,
                                     m0:m0 + 512],
                                stg[:])
            _loop.close()
    nc.compile()
    return nc


def _build_null(m=M):
    """Same I/O surface as _build, near-zero device work (for differential timing)."""
    nc = bacc.Bacc("TRN2", target_bir_lowering=False, debug=False,
                   num_devices=NCORES)
    xtp = nc.dram_tensor("xtp", [IN_F, m], F16, kind="ExternalInput").ap()
    nc.dram_tensor("qb", [IN_F // 2, OUT_F], U8, kind="ExternalInput")
    nc.dram_tensor("scl", [NG, OUT_F], F32, kind="ExternalInput")
    nc.dram_tensor("zc", [NG, OUT_F], F32R, kind="ExternalInput")
    nc.dram_tensor("gm", [IN_F, NG], F16, kind="ExternalInput")
    outT = nc.dram_tensor("outT", [OUT_F, m], F32, kind="ExternalOutput").ap()
    with tile.TileContext(nc) as tc:
        with tc.tile_pool(name="p", bufs=1) as pool:
            t = pool.tile([128, 128], F16)
            nc.sync.dma_start(t[:], xtp[0:128, 0:128])
            o = pool.tile([128, 128], F32)
            nc.vector.tensor_copy(o[:], t[:])
            nc.sync.dma_start(outT[0:128, 0:128], o[:])
    nc.compile()
    return nc


def _prep(x, qweight, qzeros, scales, m=M, ncores=NCORES):
    """Host-side layout marshaling -> per-core input maps."""
    # activations: transpose + evens-then-odds permutation within 256-blocks
    x2 = np.ascontiguousarray(x.reshape(M_TOT, IN_F))
    perm = np.empty(IN_F, dtype=np.int64)
    for t in range(NT_K):
        P, par = divmod(t, 2)
        perm[t * 128:(t + 1) * 128] = 256 * P + 2 * np.arange(128) + par
    xtp = np.ascontiguousarray(x2.T[perm]).astype(np.float16)  # [IN_F, M_TOT]

    # packed weights as byte rows: qb[k//2, n] = byte holding nibbles (2bk, 2bk+1)
    qb = np.ascontiguousarray(
        qweight.view(np.uint8).reshape(IN_F // 8, OUT_F, 4)
        .transpose(0, 2, 1).reshape(IN_F // 2, OUT_F))

    # zero-point correction constants zc[g,n] = -(scale*(z+1))
    u = qzeros.view(np.uint32)
    shifts = (4 * np.arange(8, dtype=np.uint32))[None, None, :]
    z = ((u[:, :, None] >> shifts) & np.uint32(0xF)).reshape(NG, OUT_F)
    zcv = np.ascontiguousarray((-(scales.astype(np.float64)
                                  * (z.astype(np.float64) + 1.0))).astype(np.float32))

    # one-hot group map in permuted k' order: gm[k', g] = 1 if group(k') == g
    gmv = np.zeros((IN_F, NG), dtype=np.float16)
    rows = np.arange(IN_F)
    t_idx = rows // 128
    p_idx = rows % 128
    g_idx = 2 * (t_idx // 2) + p_idx // 64
    gmv[rows, g_idx] = 1.0

    in_maps = []
    for c in range(ncores):
        in_maps.append({
            "xtp": np.ascontiguousarray(xtp[:, c * m:(c + 1) * m]),
            "qb": qb, "scl": np.ascontiguousarray(scales.astype(np.float32)),
            "zc": zcv, "gm": gmv,
        })
    return in_maps


def kernel(x, qweight, qzeros, scales):
    x = np.ascontiguousarray(np.asarray(x, dtype=np.float32))
    qweight = np.ascontiguousarray(np.asarray(qweight, dtype=np.int32))
    qzeros = np.ascontiguousarray(np.asarray(qzeros, dtype=np.int32))
    scales = np.ascontiguousarray(np.asarray(scales, dtype=np.float32))
    if "nc" not in _cache:
        _cache["nc"] = _build()
    nc = _cache["nc"]
    in_maps = _prep(x, qweight, qzeros, scales)
    results = run_bass_kernel_spmd(
        nc, in_maps, core_ids=list(range(NCORES))).results
    outs = [r["outT"] for r in results]              # each [OUT_F, M]
    full = np.concatenate(outs, axis=1)              # [OUT_F, M_TOT]
    return np.ascontiguousarray(full.T).reshape(B, SEQ, OUT_F).astype(np.float32)
